# revision 12
# baseline (speedup 1.0000x reference)
"""Multi-head attention (B=2, S=2048, E=1024, H=16, D=64) on 8 TRN2 cores.

Sharding: core c handles batch b = c//4 and head-group g = c%4 (4 heads,
256 embed cols). No cross-core communication; host slices inputs (pre-
transposed and pre-cast to f16) and gathers/normalizes outputs.

All-f16 data path (fp8 / approximate-exp variants fail the max-error gate
on rows with a dominant softmax probability). The kernel is ScalarE-bound:
128 exp ACTIVATEs of [128, 1024] at ~1.18us each. The schedule keeps the
exp stream gapless:
  - DMA order: weights, kT, qT(first 512 cols), vT, qT(rest) so the first
    score tile is ready ~10us in.
  - Upfront projections: K-ch0 (all sb), Q-ch0-sb0, V (all, progressive in
    key order, needed by the out-stage after the DEPTH lag).
  - Remaining projections (Q-ch0-sb1..3, K-ch1, Q-ch1) are interleaved one
    tile per two attention steps, using the score-psum pool so they never
    contend with the out accumulators.
  - Scores run as head pairs on disjoint 64-row PE tiles; one exp covers
    both heads; out matmuls lag DEPTH steps and accumulate the softmax
    denominator via vh's ones column (host divides).
  - K-bias dropped (softmax-invariant), V-bias applied on host.
"""

import sys

sys.path.insert(0, "/opt/trn_rl_repo")

import os

import numpy as np

if os.environ.get("JAX_PLATFORMS") == "cpu":
    # the bass program must run on the neuron cores; the axon/neuron PJRT
    # platform registers only when JAX_PLATFORMS is unset/empty
    del os.environ["JAX_PLATFORMS"]

import concourse.bass as bass  # noqa: F401
import concourse.mybir as mybir
from concourse import bacc
from concourse.tile import TileContext

B, S, E = 2, 2048, 1024
H, D = 16, 64
HPC = 4  # heads per core
COLS = HPC * D  # 256
P = 128
F32 = mybir.dt.float32
F16 = mybir.dt.float16
ET = E // P  # 8 e-tiles
JT = S // P  # 16 key tiles
NB = 512
NIQ = S // NB  # 4 query blocks
DEPTH = 8  # out-matmul lag in jt steps

_CACHED = {}


def build():
    nc = bacc.Bacc("TRN2", target_bir_lowering=False, debug=False)
    qT = nc.dram_tensor("qT", [E, S], F16, kind="ExternalInput")
    kT = nc.dram_tensor("kT", [E, S], F16, kind="ExternalInput")
    vT = nc.dram_tensor("vT", [E, S], F16, kind="ExternalInput")
    wq = nc.dram_tensor("wq", [E, COLS], F16, kind="ExternalInput")
    wk = nc.dram_tensor("wk", [E, COLS], F16, kind="ExternalInput")
    wv = nc.dram_tensor("wv", [E, COLS], F16, kind="ExternalInput")
    bq = nc.dram_tensor("bq", [P, 2], F32, kind="ExternalInput")
    # out_raw[:, (h*NIQ+iq)*NB : ...]: rows 0-63 numerator (d), row 64 denom
    out_raw = nc.dram_tensor("out_raw", [65, HPC * S], F32,
                             kind="ExternalOutput")  # [65, 8192]

    with TileContext(nc) as tc:
        with (
            tc.tile_pool(name="wp", bufs=1) as wp,
            tc.tile_pool(name="xq", bufs=ET) as xq,
            tc.tile_pool(name="xk", bufs=ET) as xk,
            tc.tile_pool(name="xv", bufs=ET) as xv,
            tc.tile_pool(name="hp", bufs=1) as hp,
            tc.tile_pool(name="pe", bufs=DEPTH + 2) as pe,
            tc.tile_pool(name="ob", bufs=4) as ob,
            tc.tile_pool(name="psS", bufs=3, space="PSUM") as psS,
            tc.tile_pool(name="psO", bufs=2, space="PSUM") as psO,
        ):
            # --- weights + bias (tiny, first so projections never stall) ---
            wq_b = wp.tile([P, ET, COLS], F16)
            wk_b = wp.tile([P, ET, COLS], F16)
            wv_b = wp.tile([P, ET, COLS], F16)
            nc.sync.dma_start(wq_b, wq.rearrange("(t p) c -> p t c", p=P))
            nc.sync.dma_start(wk_b, wk.rearrange("(t p) c -> p t c", p=P))
            nc.sync.dma_start(wv_b, wv.rearrange("(t p) c -> p t c", p=P))
            bq_t = wp.tile([P, 2], F32)
            nc.sync.dma_start(bq_t, bq[:, :])

            # --- warm the exp table set early (hides ~2.7us ACT_TABLE_LOAD) ---
            warm = wp.tile([P, 2], F32)
            nc.scalar.activation(warm, bq_t,
                                 mybir.ActivationFunctionType.Exp, scale=0.0)

            # --- activations: k first, then q sb0, then v, then q sb1-3 ---
            def x_tiles(pool, tag):
                return [pool.tile([P, S], F16, tag=tag, name=f"{tag}{t}")
                        for t in range(ET)]

            kx = x_tiles(xk, "kx")
            qx = x_tiles(xq, "qx")
            vx = x_tiles(xv, "vx")
            for et in range(ET):
                nc.sync.dma_start(kx[et], kT[et * P:(et + 1) * P, :])
            for et in range(ET):
                nc.sync.dma_start(qx[et][:, 0:NB], qT[et * P:(et + 1) * P, 0:NB])
            for et in range(ET):
                nc.sync.dma_start(vx[et], vT[et * P:(et + 1) * P, :])
            for et in range(ET):
                nc.sync.dma_start(qx[et][:, NB:], qT[et * P:(et + 1) * P, NB:])

            # --- resident head tensors ---
            qhT = hp.tile([P, 2, S], F16)  # [2 heads x 64 d, chunk, s]
            khT = hp.tile([P, 2, S], F16)
            vh_aug = hp.tile([P, JT, HPC * 65], F16)
            vh_r = vh_aug.rearrange("p j (h x) -> p j h x", x=65)
            nc.vector.memset(vh_r[:, :, :, 64:65], 1.0)

            # --- projection emitters ---
            def qk_proj_tile(pool, w_b, x, dst, bias, ch, sb):
                """One [128, 512] projection output tile: 8 matmuls + evac."""
                if pool is psO:
                    ps = pool.tile([P, NB], F32, tag="o", name="ps_p")
                    out = ps
                else:
                    ps = pool.tile([P, 2 * NB], F32, tag="s", name="ps_p")
                    out = ps[:, 0:NB]
                for et in range(ET):
                    nc.tensor.matmul(
                        out,
                        w_b[:, et, ch * P:(ch + 1) * P],
                        x[et][:, sb * NB:(sb + 1) * NB],
                        start=(et == 0),
                        stop=(et == ET - 1),
                    )
                dslc = dst[:, ch, sb * NB:(sb + 1) * NB]
                if bias is not None:
                    nc.vector.tensor_scalar_add(dslc, out, bias[:, ch:ch + 1])
                else:
                    nc.vector.tensor_copy(dslc, out)

            def v_proj_tile(scp):
                """V tiles for key tiles (2*scp, 2*scp+1): [128, 512] psum."""
                ps = psO.tile([P, NB], F32, tag="o", name="ps_v")
                for i in range(2):
                    sc = 2 * scp + i
                    for et in range(ET):
                        nc.tensor.matmul(
                            ps[:, i * COLS:(i + 1) * COLS],
                            vx[et][:, sc * P:(sc + 1) * P],
                            wv_b[:, et, :],
                            start=(et == 0),
                            stop=(et == ET - 1),
                        )
                for i in range(2):
                    sc = 2 * scp + i
                    nc.vector.tensor_copy(
                        vh_r[:, sc, :, :D],
                        ps[:, i * COLS:(i + 1) * COLS].rearrange(
                            "p (h d) -> p h d", d=D),
                    )

            # --- upfront: K ch0 (all sb), Q ch0 sb0, V (all, key order) ---
            for sb in range(NIQ):
                qk_proj_tile(psO, wk_b, kx, khT, None, 0, sb)
            qk_proj_tile(psO, wq_b, qx, qhT, bq_t, 0, 0)
            for scp in range(JT // 2):
                v_proj_tile(scp)

            # --- deferred projections, interleaved into early attention ---
            deferred = (
                [("q", 0, sb) for sb in (1, 2, 3)]
                + [("k", 1, sb) for sb in range(NIQ)]
                + [("q", 1, sb) for sb in range(NIQ)]
            )
            deferred = list(reversed(deferred))  # pop() from the front

            def emit_deferred():
                if not deferred:
                    return
                kind, ch, sb = deferred.pop()
                if kind == "q":
                    qk_proj_tile(psS, wq_b, qx, qhT, bq_t, ch, sb)
                else:
                    qk_proj_tile(psS, wk_b, kx, khT, None, ch, sb)

            # --- attention ---
            from collections import deque

            pending = deque()  # (pr, iq, jt, expT)
            ops = {}

            def emit_out(pr, iq, jt, expT):
                op0, op1 = ops[(pr, iq)]
                for hh, op in ((0, op0), (1, op1)):
                    nc.tensor.matmul(
                        op[:65, :],
                        vh_r[:, jt, 2 * pr + hh, :],
                        expT[:, hh, :],
                        start=(jt == 0),
                        stop=(jt == JT - 1),
                    )
                if jt == JT - 1:  # evacuate + store this iq's outputs
                    for hh, op in ((0, op0), (1, op1)):
                        r = (2 * pr + hh) * NIQ + iq
                        osb = ob.tile([P, NB], F32, tag="ob", name="osb")
                        nc.vector.tensor_copy(osb[:65, :], op[:65, :])
                        nc.sync.dma_start(
                            out_raw[:, r * NB:(r + 1) * NB], osb[:65, :]
                        )
                    del ops[(pr, iq)]

            steps = [(pr, iq, jt) for pr in range(2) for iq in range(NIQ)
                     for jt in range(JT)]
            for it, (pr, iq, jt) in enumerate(steps):
                if jt == 0:
                    ops[(pr, iq)] = (
                        psO.tile([P, NB], F32, tag="o", name="op0"),
                        psO.tile([P, NB], F32, tag="o", name="op1"),
                    )
                sps = psS.tile([P, 2 * NB], F32, tag="s", name="sps")
                for hh in range(2):
                    r0 = hh * D
                    nc.tensor.matmul(
                        sps[:, hh * NB:(hh + 1) * NB],
                        khT[r0:r0 + D, pr, jt * P:(jt + 1) * P],
                        qhT[r0:r0 + D, pr, iq * NB:(iq + 1) * NB],
                        start=True,
                        stop=True,
                    )
                expT = pe.tile([P, 2, NB], F16, tag="e", name="expT")
                nc.scalar.activation(
                    expT, sps, mybir.ActivationFunctionType.Exp, scale=0.125
                )
                pending.append((pr, iq, jt, expT))
                if len(pending) > DEPTH:
                    emit_out(*pending.popleft())
                if it % 2 == 1:
                    emit_deferred()
            while deferred:
                emit_deferred()
            while pending:
                emit_out(*pending.popleft())
    nc.finalize()
    return nc


def _prep_in_maps(q, k, v, wq, bq, wk, bk, wv, bv):
    bf = np.float16
    q, k, v = (np.asarray(x, np.float32) for x in (q, k, v))
    wqb, wkb, wvb = (np.asarray(x, bf) for x in (wq, wk, wv))
    bq = np.asarray(bq, np.float32)
    qT = [np.ascontiguousarray(q[b].T.astype(bf)) for b in range(B)]
    kT = [np.ascontiguousarray(k[b].T.astype(bf)) for b in range(B)]
    vT = [np.ascontiguousarray(v[b].T.astype(bf)) for b in range(B)]
    in_maps = []
    for c in range(8):
        b, g = divmod(c, 4)
        cs = slice(g * COLS, (g + 1) * COLS)
        in_maps.append(
            {
                "qT": qT[b],
                "kT": kT[b],
                "vT": vT[b],
                "wq": np.ascontiguousarray(wqb[:, cs]),
                "wk": np.ascontiguousarray(wkb[:, cs]),
                "wv": np.ascontiguousarray(wvb[:, cs]),
                "bq": np.ascontiguousarray(bq[cs].reshape(2, P).T),
            }
        )
    return in_maps


def _make_runner(nc, n_cores=8):
    """Persistent jitted shard_map runner over the prebuilt Bass module."""
    import jax
    from jax.experimental.shard_map import shard_map
    from jax.sharding import Mesh, NamedSharding, PartitionSpec
    from concourse import bass2jax

    bass2jax.install_neuronx_cc_hook()

    in_names, out_names, out_avals, zero_outs = [], [], [], []
    for alloc in nc.m.functions[0].allocations:
        if not isinstance(alloc, mybir.MemoryLocationSet):
            continue
        name = alloc.memorylocations[0].name
        if alloc.kind == "ExternalInput":
            in_names.append(name)
        elif alloc.kind == "ExternalOutput":
            shape = tuple(alloc.tensor_shape)
            dtype = mybir.dt.np(alloc.dtype)
            out_avals.append(jax.core.ShapedArray(shape, dtype))
            zero_outs.append(np.zeros((n_cores * shape[0], *shape[1:]), dtype))
            out_names.append(name)
    pid_name = nc.partition_id_tensor.name if nc.partition_id_tensor else None
    if pid_name is not None:
        in_names = [n for n in in_names if n != pid_name]
    n_params = len(in_names)
    all_names = in_names + out_names + ([pid_name] if pid_name else [])

    def _body(*args):
        operands = list(args)
        if pid_name is not None:
            operands.append(bass2jax.partition_id_tensor())
        outs = bass2jax._bass_exec_p.bind(
            *operands,
            out_avals=tuple(out_avals),
            in_names=tuple(all_names),
            out_names=tuple(out_names),
            lowering_input_output_aliases=(),
            sim_require_finite=True,
            sim_require_nnan=True,
            nc=nc,
        )
        return tuple(outs)

    devices = jax.devices()[:n_cores]
    mesh = Mesh(np.asarray(devices), ("core",))
    nio = n_params + len(out_names)
    sharded = jax.jit(
        shard_map(
            _body,
            mesh=mesh,
            in_specs=(PartitionSpec("core"),) * nio,
            out_specs=(PartitionSpec("core"),) * len(out_names),
            check_rep=False,
        ),
        keep_unused=True,
    )
    row_sharding = NamedSharding(mesh, PartitionSpec("core"))
    zeros_dev = [jax.device_put(z, row_sharding) for z in zero_outs]

    def run(in_maps):
        concat_in = [
            np.concatenate([np.asarray(m[name]) for m in in_maps], axis=0)
            for name in in_names
        ]
        out_arrs = sharded(*concat_in, *zeros_dev)
        return [
            {
                name: np.asarray(out_arrs[i]).reshape(n_cores, *out_avals[i].shape)[c]
                for i, name in enumerate(out_names)
            }
            for c in range(n_cores)
        ]

    run.sharded = sharded
    run.in_names = in_names
    run.zeros_dev = zeros_dev
    run.row_sharding = row_sharding
    return run


def get_runner():
    if "run" not in _CACHED:
        _CACHED["nc"] = build()
        _CACHED["run"] = _make_runner(_CACHED["nc"])
    return _CACHED["run"]


def kernel(q, k, v, wq, bq, wk, bk, wv, bv):
    run = get_runner()
    in_maps = _prep_in_maps(q, k, v, wq, bq, wk, bk, wv, bv)
    results = run(in_maps)

    bv = np.asarray(bv, np.float32)
    out = np.empty((B, S, E), np.float32)
    for c in range(8):
        b, g = divmod(c, 4)
        raw = results[c]["out_raw"]  # [65, 8192]
        num = raw[:64].reshape(64, HPC, S)  # [d, h, i] (NIQ*NB = S)
        den = raw[64].reshape(HPC, S)
        for h in range(HPC):
            col0 = g * COLS + h * D
            o = num[:, h, :] / den[h][None, :]
            out[b, :, col0:col0 + D] = o.T + bv[col0:col0 + D][None, :]
    return out


# revision 19
# speedup vs baseline: 1.0527x; 1.0527x over previous
"""Multi-head attention (B=2, S=2048, E=1024, H=16, D=64) on 8 TRN2 cores.

Sharding: core c handles batch b = c//4 and head-group g = c%4 (4 heads,
256 embed cols). No cross-core communication; host slices inputs (pre-
transposed and pre-cast to f16) and gathers/normalizes outputs.

All-f16 data path (fp8 / approximate-exp variants fail the max-error gate
on rows with a dominant softmax probability). The kernel is ScalarE-bound:
128 exp ACTIVATEs of [128, 1024] at ~1.18us each. The schedule keeps the
exp stream gapless:
  - DMA order: weights, kT, qT(first 512 cols), vT, qT(rest) so the first
    score tile is ready ~10us in.
  - Upfront projections: K-ch0 (all sb), Q-ch0-sb0, V (all, progressive in
    key order, needed by the out-stage after the DEPTH lag).
  - Remaining projections (Q-ch0-sb1..3, K-ch1, Q-ch1) are interleaved one
    tile per two attention steps, using the score-psum pool so they never
    contend with the out accumulators.
  - Scores run as head pairs on disjoint 64-row PE tiles; one exp covers
    both heads; out matmuls lag DEPTH steps and accumulate the softmax
    denominator via vh's ones column (host divides).
  - K-bias dropped (softmax-invariant), V-bias applied on host.
"""

import sys

sys.path.insert(0, "/opt/trn_rl_repo")

import os

import numpy as np

if os.environ.get("JAX_PLATFORMS") == "cpu":
    # the bass program must run on the neuron cores; the axon/neuron PJRT
    # platform registers only when JAX_PLATFORMS is unset/empty
    del os.environ["JAX_PLATFORMS"]

import concourse.bass as bass  # noqa: F401
import concourse.mybir as mybir
from concourse import bacc
from concourse.tile import TileContext

B, S, E = 2, 2048, 1024
H, D = 16, 64
HPC = 4  # heads per core
COLS = HPC * D  # 256
P = 128
F32 = mybir.dt.float32
F16 = mybir.dt.float16
ET = E // P  # 8 e-tiles
JT = S // P  # 16 key tiles
NB = 512
NIQ = S // NB  # 4 query blocks
DEPTH1 = 6  # steady-state out-matmul lag (jt steps)
PE_BUFS = 30  # expT pool depth (peak backlog while iq0's outs wait on V)

_CACHED = {}


def build():
    nc = bacc.Bacc("TRN2", target_bir_lowering=False, debug=False)
    qT = nc.dram_tensor("qT", [E, S], F16, kind="ExternalInput")
    kT = nc.dram_tensor("kT", [E, S], F16, kind="ExternalInput")
    vT = nc.dram_tensor("vT", [E, S], F16, kind="ExternalInput")
    wq = nc.dram_tensor("wq", [E, COLS], F16, kind="ExternalInput")
    wk = nc.dram_tensor("wk", [E, COLS], F16, kind="ExternalInput")
    wv = nc.dram_tensor("wv", [E, COLS], F16, kind="ExternalInput")
    bq = nc.dram_tensor("bq", [P, 2], F32, kind="ExternalInput")
    # out_raw[:, (h*NIQ+iq)*NB : ...]: rows 0-63 numerator (d), row 64 denom
    out_raw = nc.dram_tensor("out_raw", [65, HPC * S], F32,
                             kind="ExternalOutput")  # [65, 8192]

    with TileContext(nc) as tc:
        with (
            tc.tile_pool(name="wp", bufs=1) as wp,
            tc.tile_pool(name="xp", bufs=1) as xp,
            tc.tile_pool(name="hp", bufs=1) as hp,
            tc.tile_pool(name="pe", bufs=PE_BUFS) as pe,
            tc.tile_pool(name="ob", bufs=4) as ob,
            tc.tile_pool(name="psS", bufs=2, space="PSUM") as psS,
            tc.tile_pool(name="psO", bufs=2, space="PSUM") as psO,
        ):
            # --- weights wk/wq + bias first; wv deferred until after kT/qT ---
            wq_b = wp.tile([P, ET, COLS], F16)
            wk_b = wp.tile([P, ET, COLS], F16)
            wv_b = wp.tile([P, ET, COLS], F16)
            bq_t = wp.tile([P, 2], F32)
            kxt = xp.tile([P, ET, S], F16)
            qxt = xp.tile([P, ET, S], F16)
            vxt = xp.tile([P, ET, S], F16)
            kx = [kxt[:, et, :] for et in range(ET)]
            qx = [qxt[:, et, :] for et in range(ET)]
            vx = [vxt[:, et, :] for et in range(ET)]

            def x_chunk(dst, src, c0, c1):
                nc.sync.dma_start(
                    dst[:, :, c0:c1],
                    src.rearrange("(t p) s -> p t s", p=P)[:, :, c0:c1],
                )

            # DMA order tuned so each consumer's data lands just in time:
            # first exp needs wk/wq + kT sb0 + qT sb0; scores jt4/8/12 need
            # kT sb1-3; V tiles (needed from step DEPTH0 on) need wv + vT.
            nc.sync.dma_start(wk_b, wk.rearrange("(t p) c -> p t c", p=P))
            nc.sync.dma_start(wq_b, wq.rearrange("(t p) c -> p t c", p=P))
            nc.sync.dma_start(bq_t, bq[:, :])
            x_chunk(kxt, kT, 0, NB)
            x_chunk(qxt, qT, 0, NB)
            x_chunk(kxt, kT, NB, 2 * NB)
            x_chunk(kxt, kT, 2 * NB, 3 * NB)
            x_chunk(kxt, kT, 3 * NB, 4 * NB)
            x_chunk(qxt, qT, NB, 2 * NB)
            nc.sync.dma_start(wv_b, wv.rearrange("(t p) c -> p t c", p=P))
            x_chunk(vxt, vT, 0, 2 * NB)
            x_chunk(vxt, vT, 2 * NB, 4 * NB)
            x_chunk(qxt, qT, 2 * NB, 3 * NB)
            x_chunk(qxt, qT, 3 * NB, 4 * NB)

            # --- warm the exp table set early (hides ~2.7us ACT_TABLE_LOAD) ---
            warm = wp.tile([P, 2], F32)
            nc.scalar.activation(warm, bq_t,
                                 mybir.ActivationFunctionType.Exp, scale=0.0)

            # --- resident head tensors ---
            qhT = hp.tile([P, 2, S], F16)  # [2 heads x 64 d, chunk, s]
            khT = hp.tile([P, 2, S], F16)
            vh_aug = hp.tile([P, JT, HPC * 65], F16)
            vh_r = vh_aug.rearrange("p j (h x) -> p j h x", x=65)
            nc.vector.memset(vh_r[:, :, :, 64:65], 1.0)

            # --- projection emitters ---
            def qk_proj_tile(tag, w_b, x, dst, bias, ch, sb):
                """One [128, 512] projection output tile: 8 matmuls + evac."""
                ps = psO.tile([P, NB], F32, tag=tag, name=f"ps_{tag}")
                for et in range(ET):
                    nc.tensor.matmul(
                        ps,
                        w_b[:, et, ch * P:(ch + 1) * P],
                        x[et][:, sb * NB:(sb + 1) * NB],
                        start=(et == 0),
                        stop=(et == ET - 1),
                    )
                dslc = dst[:, ch, sb * NB:(sb + 1) * NB]
                if bias is not None:
                    nc.vector.tensor_scalar_add(dslc, ps, bias[:, ch:ch + 1])
                else:
                    nc.vector.tensor_copy(dslc, ps)

            def v_proj_tile(scp):
                """V tiles for key tiles (2*scp, 2*scp+1): [128, 512] psum."""
                ps = psO.tile([P, NB], F32, tag="v", name="ps_v")
                for i in range(2):
                    sc = 2 * scp + i
                    for et in range(ET):
                        nc.tensor.matmul(
                            ps[:, i * COLS:(i + 1) * COLS],
                            vx[et][:, sc * P:(sc + 1) * P],
                            wv_b[:, et, :],
                            start=(et == 0),
                            stop=(et == ET - 1),
                        )
                for i in range(2):
                    sc = 2 * scp + i
                    nc.vector.tensor_copy(
                        vh_r[:, sc, :, :D],
                        ps[:, i * COLS:(i + 1) * COLS].rearrange(
                            "p (h d) -> p h d", d=D),
                    )

            # --- upfront: just K ch0 sb0 + Q ch0 sb0 (gates the first exp) ---
            qk_proj_tile("o", wk_b, kx, khT, None, 0, 0)
            qk_proj_tile("o", wq_b, qx, qhT, bq_t, 0, 0)

            # --- everything else is deferred, interleaved into attention.
            # slot = step after which the tile is emitted, chosen to land
            # just after its input DMA completes (PE is in-order: emitting
            # before the DMA lands would block the score stream).
            deferred = (
                [(2, ("k", 0, 1)), (4, ("k", 0, 2)), (7, ("k", 0, 3)),
                 (9, ("q", 0, 1))]
                + [(15 + 3 * scp, ("v", scp, None)) for scp in range(8)]
                + [(38, ("q", 0, 2)), (40, ("q", 0, 3)),
                   (42, ("k", 1, 0)), (44, ("k", 1, 1)),
                   (46, ("k", 1, 2)), (48, ("k", 1, 3)),
                   (50, ("q", 1, 0)), (52, ("q", 1, 1)),
                   (54, ("q", 1, 2)), (56, ("q", 1, 3))]
            )
            sched = {}
            for (s, item) in deferred:
                sched.setdefault(s, []).append(item)

            def emit_deferred(it):
                for item in sched.pop(it, ()):
                    kind, a, b = item
                    if kind == "v":
                        v_proj_tile(a)
                    elif kind == "q":
                        qk_proj_tile("v", wq_b, qx, qhT, bq_t, a, b)
                    else:
                        qk_proj_tile("v", wk_b, kx, khT, None, a, b)

            # --- attention ---
            from collections import deque

            pending = deque()  # (pr, iq, jt, expT)
            ops = {}

            def emit_out(pr, iq, jt, expT):
                op0, op1 = ops[(pr, iq)]
                for hh, op in ((0, op0), (1, op1)):
                    nc.tensor.matmul(
                        op[:65, :],
                        vh_r[:, jt, 2 * pr + hh, :],
                        expT[:, hh, :],
                        start=(jt == 0),
                        stop=(jt == JT - 1),
                    )
                if jt == JT - 1:  # evacuate + store this iq's outputs
                    for hh, op in ((0, op0), (1, op1)):
                        r = (2 * pr + hh) * NIQ + iq
                        osb = ob.tile([P, NB], F32, tag="ob", name="osb")
                        nc.vector.tensor_copy(osb[:65, :], op[:65, :])
                        nc.sync.dma_start(
                            out_raw[:, r * NB:(r + 1) * NB], osb[:65, :]
                        )
                    del ops[(pr, iq)]

            # out(pr0,iq0,jt) must wait for its V tile (DMA-gated); later
            # outs just lag DEPTH1 steps. Pops are capped at 2/step so a
            # backlog drains without swamping a single ACT period.
            def ready_step(it0, pr, iq, jt):
                if it0 < JT:
                    return 19 + 3 * (jt // 2)
                return it0 + DEPTH1

            steps = [(pr, iq, jt) for pr in range(2) for iq in range(NIQ)
                     for jt in range(JT)]
            for it, (pr, iq, jt) in enumerate(steps):
                if jt == 0:
                    ops[(pr, iq)] = (
                        psO.tile([P, NB], F32, tag="o", name="op0"),
                        psO.tile([P, NB], F32, tag="o", name="op1"),
                    )
                sps = psS.tile([P, 2 * NB], F32, tag="s", name="sps")
                for hh in range(2):
                    r0 = hh * D
                    nc.tensor.matmul(
                        sps[:, hh * NB:(hh + 1) * NB],
                        khT[r0:r0 + D, pr, jt * P:(jt + 1) * P],
                        qhT[r0:r0 + D, pr, iq * NB:(iq + 1) * NB],
                        start=True,
                        stop=True,
                    )
                expT = pe.tile([P, 2, NB], F16, tag="e", name="expT")
                nc.scalar.activation(
                    expT, sps, mybir.ActivationFunctionType.Exp, scale=0.125
                )
                pending.append((it, pr, iq, jt, expT))
                for _ in range(2):
                    if pending and ready_step(*pending[0][:3],
                                              pending[0][3]) <= it:
                        emit_out(*pending.popleft()[1:])
                    else:
                        break
                emit_deferred(it)
            while pending:
                emit_out(*pending.popleft()[1:])
    nc.finalize()
    return nc


def _prep_in_maps(q, k, v, wq, bq, wk, bk, wv, bv):
    bf = np.float16
    q, k, v = (np.asarray(x, np.float32) for x in (q, k, v))
    wqb, wkb, wvb = (np.asarray(x, bf) for x in (wq, wk, wv))
    bq = np.asarray(bq, np.float32)
    qT = [np.ascontiguousarray(q[b].T.astype(bf)) for b in range(B)]
    kT = [np.ascontiguousarray(k[b].T.astype(bf)) for b in range(B)]
    vT = [np.ascontiguousarray(v[b].T.astype(bf)) for b in range(B)]
    in_maps = []
    for c in range(8):
        b, g = divmod(c, 4)
        cs = slice(g * COLS, (g + 1) * COLS)
        in_maps.append(
            {
                "qT": qT[b],
                "kT": kT[b],
                "vT": vT[b],
                "wq": np.ascontiguousarray(wqb[:, cs]),
                "wk": np.ascontiguousarray(wkb[:, cs]),
                "wv": np.ascontiguousarray(wvb[:, cs]),
                "bq": np.ascontiguousarray(bq[cs].reshape(2, P).T),
            }
        )
    return in_maps


def _make_runner(nc, n_cores=8):
    """Persistent jitted shard_map runner over the prebuilt Bass module."""
    import jax
    from jax.experimental.shard_map import shard_map
    from jax.sharding import Mesh, NamedSharding, PartitionSpec
    from concourse import bass2jax

    bass2jax.install_neuronx_cc_hook()

    in_names, out_names, out_avals, zero_outs = [], [], [], []
    for alloc in nc.m.functions[0].allocations:
        if not isinstance(alloc, mybir.MemoryLocationSet):
            continue
        name = alloc.memorylocations[0].name
        if alloc.kind == "ExternalInput":
            in_names.append(name)
        elif alloc.kind == "ExternalOutput":
            shape = tuple(alloc.tensor_shape)
            dtype = mybir.dt.np(alloc.dtype)
            out_avals.append(jax.core.ShapedArray(shape, dtype))
            zero_outs.append(np.zeros((n_cores * shape[0], *shape[1:]), dtype))
            out_names.append(name)
    pid_name = nc.partition_id_tensor.name if nc.partition_id_tensor else None
    if pid_name is not None:
        in_names = [n for n in in_names if n != pid_name]
    n_params = len(in_names)
    all_names = in_names + out_names + ([pid_name] if pid_name else [])

    def _body(*args):
        operands = list(args)
        if pid_name is not None:
            operands.append(bass2jax.partition_id_tensor())
        outs = bass2jax._bass_exec_p.bind(
            *operands,
            out_avals=tuple(out_avals),
            in_names=tuple(all_names),
            out_names=tuple(out_names),
            lowering_input_output_aliases=(),
            sim_require_finite=True,
            sim_require_nnan=True,
            nc=nc,
        )
        return tuple(outs)

    devices = jax.devices()[:n_cores]
    mesh = Mesh(np.asarray(devices), ("core",))
    nio = n_params + len(out_names)
    sharded = jax.jit(
        shard_map(
            _body,
            mesh=mesh,
            in_specs=(PartitionSpec("core"),) * nio,
            out_specs=(PartitionSpec("core"),) * len(out_names),
            check_rep=False,
        ),
        keep_unused=True,
    )
    row_sharding = NamedSharding(mesh, PartitionSpec("core"))
    zeros_dev = [jax.device_put(z, row_sharding) for z in zero_outs]

    def run(in_maps):
        concat_in = [
            np.concatenate([np.asarray(m[name]) for m in in_maps], axis=0)
            for name in in_names
        ]
        out_arrs = sharded(*concat_in, *zeros_dev)
        return [
            {
                name: np.asarray(out_arrs[i]).reshape(n_cores, *out_avals[i].shape)[c]
                for i, name in enumerate(out_names)
            }
            for c in range(n_cores)
        ]

    run.sharded = sharded
    run.in_names = in_names
    run.zeros_dev = zeros_dev
    run.row_sharding = row_sharding
    return run


def get_runner():
    if "run" not in _CACHED:
        _CACHED["nc"] = build()
        _CACHED["run"] = _make_runner(_CACHED["nc"])
    return _CACHED["run"]


def kernel(q, k, v, wq, bq, wk, bk, wv, bv):
    run = get_runner()
    in_maps = _prep_in_maps(q, k, v, wq, bq, wk, bk, wv, bv)
    results = run(in_maps)

    bv = np.asarray(bv, np.float32)
    out = np.empty((B, S, E), np.float32)
    for c in range(8):
        b, g = divmod(c, 4)
        raw = results[c]["out_raw"]  # [65, 8192]
        num = raw[:64].reshape(64, HPC, S)  # [d, h, i] (NIQ*NB = S)
        den = raw[64].reshape(HPC, S)
        for h in range(HPC):
            col0 = g * COLS + h * D
            o = num[:, h, :] / den[h][None, :]
            out[b, :, col0:col0 + D] = o.T + bv[col0:col0 + D][None, :]
    return out


# revision 26
# speedup vs baseline: 1.1157x; 1.0598x over previous
"""Multi-head attention (B=2, S=2048, E=1024, H=16, D=64) on 8 TRN2 cores.

Sharding: core c handles batch b = c//4 and head-group g = c%4 (4 heads,
256 embed cols). No cross-core communication; host slices inputs (pre-
transposed and pre-cast to f16) and gathers/normalizes outputs.

All-f16 data path (fp8 / approximate-exp variants fail the max-error gate
on rows with a dominant softmax probability). The kernel is ScalarE-bound:
128 exp ACTIVATEs of [128, 1024] at ~1.18us each. The schedule keeps the
exp stream gapless:
  - DMA order: weights, kT, qT(first 512 cols), vT, qT(rest) so the first
    score tile is ready ~10us in.
  - Upfront projections: K-ch0 (all sb), Q-ch0-sb0, V (all, progressive in
    key order, needed by the out-stage after the DEPTH lag).
  - Remaining projections (Q-ch0-sb1..3, K-ch1, Q-ch1) are interleaved one
    tile per two attention steps, using the score-psum pool so they never
    contend with the out accumulators.
  - Scores run as head pairs on disjoint 64-row PE tiles; one exp covers
    both heads; out matmuls lag DEPTH steps and accumulate the softmax
    denominator via vh's ones column (host divides).
  - K-bias dropped (softmax-invariant), V-bias applied on host.
"""

import sys

sys.path.insert(0, "/opt/trn_rl_repo")

import os

import numpy as np

if os.environ.get("JAX_PLATFORMS") == "cpu":
    # the bass program must run on the neuron cores; the axon/neuron PJRT
    # platform registers only when JAX_PLATFORMS is unset/empty
    del os.environ["JAX_PLATFORMS"]

from collections import deque

import concourse.bass as bass  # noqa: F401
import concourse.mybir as mybir
from concourse import bacc
from concourse.tile import TileContext

B, S, E = 2, 2048, 1024
H, D = 16, 64
HPC = 4  # heads per core
COLS = HPC * D  # 256
P = 128
F32 = mybir.dt.float32
F16 = mybir.dt.float16
ET = E // P  # 8 e-tiles
JT = S // P  # 16 key tiles
NB = 512
NIQ = S // NB  # 4 query blocks
DEPTH1 = 6  # steady-state out-matmul lag (jt steps)
PE_BUFS = 30  # expT pool depth (peak backlog while iq0's outs wait on V)

_CACHED = {}


def build():
    nc = bacc.Bacc("TRN2", target_bir_lowering=False, debug=False)
    # host pre-arranges x as [p, chunk(4), et(8), 512] so each chunk DMA is a
    # flat [128 x 8KB] contiguous transfer (descriptor-gen is ~rows-bound),
    # and weights as [p, ch(2), et(8), 128] so ch0 can load first.
    qT = nc.dram_tensor("qT", [P, NIQ * ET * NB], F16, kind="ExternalInput")
    kT = nc.dram_tensor("kT", [P, NIQ * ET * NB], F16, kind="ExternalInput")
    vT = nc.dram_tensor("vT", [P, NIQ * ET * NB], F16, kind="ExternalInput")
    wq = nc.dram_tensor("wq", [P, 2 * ET * P], F16, kind="ExternalInput")
    wk = nc.dram_tensor("wk", [P, 2 * ET * P], F16, kind="ExternalInput")
    wv = nc.dram_tensor("wv", [P, ET * COLS], F16, kind="ExternalInput")
    bq = nc.dram_tensor("bq", [P, 2], F32, kind="ExternalInput")
    # out_raw[:, (h*NIQ+iq)*NB : ...]: rows 0-63 numerator (d), row 64 denom
    out_raw = nc.dram_tensor("out_raw", [65, HPC * S], F32,
                             kind="ExternalOutput")  # [65, 8192]

    with TileContext(nc) as tc:
        with (
            tc.tile_pool(name="wp", bufs=1) as wp,
            tc.tile_pool(name="xp", bufs=1) as xp,
            tc.tile_pool(name="hp", bufs=1) as hp,
            tc.tile_pool(name="pe", bufs=PE_BUFS) as pe,
            tc.tile_pool(name="ob", bufs=4) as ob,
            tc.tile_pool(name="psS", bufs=2, space="PSUM") as psS,
            tc.tile_pool(name="psO", bufs=2, space="PSUM") as psO,
        ):
            # --- weights wk/wq ch0 + bias first; rest lands just in time ---
            wq_b = wp.tile([P, 2, ET, P], F16)  # [p, ch, et, m]
            wk_b = wp.tile([P, 2, ET, P], F16)
            wv_b = wp.tile([P, ET, COLS], F16)
            bq_t = wp.tile([P, 2], F32)
            # x: [p, chunk(sb), et, 512]
            kxt = xp.tile([P, NIQ, ET, NB], F16)
            qxt = xp.tile([P, NIQ, ET, NB], F16)
            vxt = xp.tile([P, NIQ, ET, NB], F16)

            def x_chunk(dst, src, c0, c1):
                nc.sync.dma_start(dst[:, c0:c1],
                                  src[:, c0 * ET * NB:c1 * ET * NB])

            CH = ET * P  # flat weight cols per chunk
            nc.sync.dma_start(wk_b[:, 0], wk[:, 0:CH])
            nc.sync.dma_start(wq_b[:, 0], wq[:, 0:CH])
            nc.sync.dma_start(bq_t, bq[:, :])
            x_chunk(kxt, kT, 0, 1)
            x_chunk(qxt, qT, 0, 1)
            x_chunk(kxt, kT, 1, 2)
            x_chunk(kxt, kT, 2, 4)
            x_chunk(qxt, qT, 1, 2)
            nc.sync.dma_start(wv_b, wv[:, :])
            x_chunk(vxt, vT, 0, 2)
            x_chunk(vxt, vT, 2, 4)
            x_chunk(qxt, qT, 2, 3)
            x_chunk(qxt, qT, 3, 4)
            nc.sync.dma_start(wk_b[:, 1], wk[:, CH:])
            nc.sync.dma_start(wq_b[:, 1], wq[:, CH:])

            # --- warm the exp table set early (hides ~2.7us ACT_TABLE_LOAD) ---
            warm = wp.tile([P, 2], F32)
            nc.scalar.activation(warm, bq_t,
                                 mybir.ActivationFunctionType.Exp, scale=0.0)

            # --- resident head tensors ---
            qhT = hp.tile([P, 2, S], F16)  # [2 heads x 64 d, chunk, s]
            khT = hp.tile([P, 2, S], F16)
            vh_aug = hp.tile([P, JT, HPC * 65], F16)
            vh_r = vh_aug.rearrange("p j (h x) -> p j h x", x=65)
            nc.vector.memset(vh_r[:, :, :, 64:65], 1.0)

            # --- projection emitters (generators yielding ~430ns chunks) ---
            def qk_proj_tile(tag, w_b, dst, bias, ch, sb):
                """One [128, 512] projection output tile: 8 matmuls + evac."""
                x = kxt if dst is khT else qxt
                ps = psO.tile([P, NB], F32, tag=tag, name=f"ps_{tag}")
                for et in range(ET):
                    nc.tensor.matmul(
                        ps,
                        w_b[:, ch, et, :],
                        x[:, sb, et, :],
                        start=(et == 0),
                        stop=(et == ET - 1),
                    )
                    if et % 2 == 1 and et < ET - 1:
                        yield
                dslc = dst[:, ch, sb * NB:(sb + 1) * NB]
                if bias is not None:
                    nc.vector.tensor_scalar_add(dslc, ps, bias[:, ch:ch + 1])
                else:
                    nc.vector.tensor_copy(dslc, ps)
                yield

            def v_proj_tile(scp):
                """V tiles for key tiles (2*scp, 2*scp+1): [128, 512] psum."""
                ps = psO.tile([P, NB], F32, tag="v", name="ps_v")
                for i in range(2):
                    sc = 2 * scp + i
                    for et in range(ET):
                        nc.tensor.matmul(
                            ps[:, i * COLS:(i + 1) * COLS],
                            vxt[:, sc // 4, et, (sc % 4) * P:(sc % 4 + 1) * P],
                            wv_b[:, et, :],
                            start=(et == 0),
                            stop=(et == ET - 1),
                        )
                        if et % 4 == 3 and (i, et) != (1, ET - 1):
                            yield
                for i in range(2):
                    sc = 2 * scp + i
                    nc.vector.tensor_copy(
                        vh_r[:, sc, :, :D],
                        ps[:, i * COLS:(i + 1) * COLS].rearrange(
                            "p (h d) -> p h d", d=D),
                    )
                yield

            # --- upfront: just K ch0 sb0 + Q ch0 sb0 (gates the first exp) ---
            for _ in qk_proj_tile("o", wk_b, khT, None, 0, 0):
                pass
            for _ in qk_proj_tile("o", wq_b, qhT, bq_t, 0, 0):
                pass

            # --- everything else is deferred, interleaved into attention as
            # small chunks (PE is in-order: a whole 1.7us tile between two
            # score pairs would stall the exp stream). slot = earliest step.
            def mk(kind, a, b):
                if kind == "v":
                    return v_proj_tile(a)
                if kind == "q":
                    return qk_proj_tile("v", wq_b, qhT, bq_t, a, b)
                return qk_proj_tile("v", wk_b, khT, None, a, b)

            deferred = deque(
                [(2, ("k", 0, 1)), (4, ("k", 0, 2)), (7, ("k", 0, 3)),
                 (9, ("q", 0, 1))]
                + [(15 + 3 * scp, ("v", scp, None)) for scp in range(8)]
                + [(40, ("q", 0, 2)), (43, ("q", 0, 3)),
                   (46, ("k", 1, 0)), (49, ("k", 1, 1)),
                   (52, ("k", 1, 2)), (55, ("k", 1, 3)),
                   (58, ("q", 1, 0)), (61, ("q", 1, 1)),
                   (64, ("q", 1, 2)), (67, ("q", 1, 3))]
            )
            live_gen = []

            def emit_deferred(it):
                # one ~430ns chunk per step keeps PE ahead of ScalarE
                if live_gen:
                    try:
                        next(live_gen[0])
                    except StopIteration:
                        live_gen.pop(0)
                    return
                if deferred and deferred[0][0] <= it:
                    _, (kind, a, b) = deferred.popleft()
                    live_gen.append(mk(kind, a, b))
                    emit_deferred(it)

            # --- attention ---
            pending = deque()  # (it, pr, iq, jt, expT)
            ops = {}

            def emit_out(pr, iq, jt, expT):
                op0, op1 = ops[(pr, iq)]
                for hh, op in ((0, op0), (1, op1)):
                    nc.tensor.matmul(
                        op[:65, :],
                        vh_r[:, jt, 2 * pr + hh, :],
                        expT[:, hh, :],
                        start=(jt == 0),
                        stop=(jt == JT - 1),
                    )
                if jt == JT - 1:  # evacuate + store this iq's outputs
                    for hh, op in ((0, op0), (1, op1)):
                        r = (2 * pr + hh) * NIQ + iq
                        osb = ob.tile([P, NB], F32, tag="ob", name="osb")
                        nc.vector.tensor_copy(osb[:65, :], op[:65, :])
                        nc.sync.dma_start(
                            out_raw[:, r * NB:(r + 1) * NB], osb[:65, :]
                        )
                    del ops[(pr, iq)]

            # out(pr0,iq0,jt) must wait for its V tile (DMA-gated); later
            # outs just lag DEPTH1 steps. Pops are capped at 2/step so a
            # backlog drains without swamping a single ACT period.
            def ready_step(it0, pr, iq, jt):
                if it0 < JT:
                    return 19 + 3 * (jt // 2)
                return it0 + DEPTH1

            steps = [(pr, iq, jt) for pr in range(2) for iq in range(NIQ)
                     for jt in range(JT)]
            for it, (pr, iq, jt) in enumerate(steps):
                if jt == 0:
                    ops[(pr, iq)] = (
                        psO.tile([P, NB], F32, tag="o", name="op0"),
                        psO.tile([P, NB], F32, tag="o", name="op1"),
                    )
                sps = psS.tile([P, 2 * NB], F32, tag="s", name="sps")
                for hh in range(2):
                    r0 = hh * D
                    nc.tensor.matmul(
                        sps[:, hh * NB:(hh + 1) * NB],
                        khT[r0:r0 + D, pr, jt * P:(jt + 1) * P],
                        qhT[r0:r0 + D, pr, iq * NB:(iq + 1) * NB],
                        start=True,
                        stop=True,
                    )
                expT = pe.tile([P, 2, NB], F16, tag="e", name="expT")
                nc.scalar.activation(
                    expT, sps, mybir.ActivationFunctionType.Exp, scale=0.125
                )
                pending.append((it, pr, iq, jt, expT))
                max_pops = 2 if it < 120 else 4
                for _ in range(max_pops):
                    if pending and ready_step(*pending[0][:3],
                                              pending[0][3]) <= it:
                        emit_out(*pending.popleft()[1:])
                    else:
                        break
                emit_deferred(it)
            while live_gen or deferred:
                emit_deferred(1 << 30)
            while pending:
                emit_out(*pending.popleft()[1:])
    nc.finalize()
    return nc


def _prep_in_maps(q, k, v, wq, bq, wk, bk, wv, bv):
    bf = np.float16
    q, k, v = (np.asarray(x, np.float32) for x in (q, k, v))
    wqb, wkb, wvb = (np.asarray(x, bf) for x in (wq, wk, wv))
    bq = np.asarray(bq, np.float32)

    def prep_x(x):
        # [S, E] -> [p, sb, et, nb] flattened to [P, NIQ*ET*NB]
        a = x.astype(bf).reshape(NIQ, NB, ET, P).transpose(3, 0, 2, 1)
        return np.ascontiguousarray(a.reshape(P, NIQ * ET * NB))

    def prep_wqk(w, cs):
        # [E, 256] -> [p, ch, et, m] flattened
        a = w[:, cs].reshape(ET, P, 2, P).transpose(1, 2, 0, 3)
        return np.ascontiguousarray(a.reshape(P, 2 * ET * P))

    def prep_wv(w, cs):
        a = w[:, cs].reshape(ET, P, COLS).transpose(1, 0, 2)
        return np.ascontiguousarray(a.reshape(P, ET * COLS))

    qT = [prep_x(q[b]) for b in range(B)]
    kT = [prep_x(k[b]) for b in range(B)]
    vT = [prep_x(v[b]) for b in range(B)]
    in_maps = []
    for c in range(8):
        b, g = divmod(c, 4)
        cs = slice(g * COLS, (g + 1) * COLS)
        in_maps.append(
            {
                "qT": qT[b],
                "kT": kT[b],
                "vT": vT[b],
                "wq": prep_wqk(wqb, cs),
                "wk": prep_wqk(wkb, cs),
                "wv": prep_wv(wvb, cs),
                "bq": np.ascontiguousarray(bq[cs].reshape(2, P).T),
            }
        )
    return in_maps


def _make_runner(nc, n_cores=8):
    """Persistent jitted shard_map runner over the prebuilt Bass module."""
    import jax
    from jax.experimental.shard_map import shard_map
    from jax.sharding import Mesh, NamedSharding, PartitionSpec
    from concourse import bass2jax

    bass2jax.install_neuronx_cc_hook()

    in_names, out_names, out_avals, zero_outs = [], [], [], []
    for alloc in nc.m.functions[0].allocations:
        if not isinstance(alloc, mybir.MemoryLocationSet):
            continue
        name = alloc.memorylocations[0].name
        if alloc.kind == "ExternalInput":
            in_names.append(name)
        elif alloc.kind == "ExternalOutput":
            shape = tuple(alloc.tensor_shape)
            dtype = mybir.dt.np(alloc.dtype)
            out_avals.append(jax.core.ShapedArray(shape, dtype))
            zero_outs.append(np.zeros((n_cores * shape[0], *shape[1:]), dtype))
            out_names.append(name)
    pid_name = nc.partition_id_tensor.name if nc.partition_id_tensor else None
    if pid_name is not None:
        in_names = [n for n in in_names if n != pid_name]
    n_params = len(in_names)
    all_names = in_names + out_names + ([pid_name] if pid_name else [])

    def _body(*args):
        operands = list(args)
        if pid_name is not None:
            operands.append(bass2jax.partition_id_tensor())
        outs = bass2jax._bass_exec_p.bind(
            *operands,
            out_avals=tuple(out_avals),
            in_names=tuple(all_names),
            out_names=tuple(out_names),
            lowering_input_output_aliases=(),
            sim_require_finite=True,
            sim_require_nnan=True,
            nc=nc,
        )
        return tuple(outs)

    devices = jax.devices()[:n_cores]
    mesh = Mesh(np.asarray(devices), ("core",))
    nio = n_params + len(out_names)
    sharded = jax.jit(
        shard_map(
            _body,
            mesh=mesh,
            in_specs=(PartitionSpec("core"),) * nio,
            out_specs=(PartitionSpec("core"),) * len(out_names),
            check_rep=False,
        ),
        keep_unused=True,
    )
    row_sharding = NamedSharding(mesh, PartitionSpec("core"))
    zeros_dev = [jax.device_put(z, row_sharding) for z in zero_outs]

    def run(in_maps):
        concat_in = [
            np.concatenate([np.asarray(m[name]) for m in in_maps], axis=0)
            for name in in_names
        ]
        out_arrs = sharded(*concat_in, *zeros_dev)
        return [
            {
                name: np.asarray(out_arrs[i]).reshape(n_cores, *out_avals[i].shape)[c]
                for i, name in enumerate(out_names)
            }
            for c in range(n_cores)
        ]

    run.sharded = sharded
    run.in_names = in_names
    run.zeros_dev = zeros_dev
    run.row_sharding = row_sharding
    return run


def get_runner():
    if "run" not in _CACHED:
        _CACHED["nc"] = build()
        _CACHED["run"] = _make_runner(_CACHED["nc"])
    return _CACHED["run"]


def kernel(q, k, v, wq, bq, wk, bk, wv, bv):
    run = get_runner()
    in_maps = _prep_in_maps(q, k, v, wq, bq, wk, bk, wv, bv)
    results = run(in_maps)

    bv = np.asarray(bv, np.float32)
    out = np.empty((B, S, E), np.float32)
    for c in range(8):
        b, g = divmod(c, 4)
        raw = results[c]["out_raw"]  # [65, 8192]
        num = raw[:64].reshape(64, HPC, S)  # [d, h, i] (NIQ*NB = S)
        den = raw[64].reshape(HPC, S)
        for h in range(HPC):
            col0 = g * COLS + h * D
            o = num[:, h, :] / den[h][None, :]
            out[b, :, col0:col0 + D] = o.T + bv[col0:col0 + D][None, :]
    return out


# revision 32
# speedup vs baseline: 1.1251x; 1.0085x over previous
"""Multi-head attention (B=2, S=2048, E=1024, H=16, D=64) on 8 TRN2 cores.

Sharding: core c handles batch b = c//4 and head-group g = c%4 (4 heads,
256 embed cols). No cross-core communication; host slices inputs (pre-
transposed and pre-cast to f16) and gathers/normalizes outputs.

All-f16 data path (fp8 / approximate-exp variants fail the max-error gate
on rows with a dominant softmax probability). The kernel is ScalarE-bound:
128 exp ACTIVATEs of [128, 1024] at ~1.18us each. The schedule keeps the
exp stream gapless:
  - DMA order: weights, kT, qT(first 512 cols), vT, qT(rest) so the first
    score tile is ready ~10us in.
  - Upfront projections: K-ch0 (all sb), Q-ch0-sb0, V (all, progressive in
    key order, needed by the out-stage after the DEPTH lag).
  - Remaining projections (Q-ch0-sb1..3, K-ch1, Q-ch1) are interleaved one
    tile per two attention steps, using the score-psum pool so they never
    contend with the out accumulators.
  - Scores run as head pairs on disjoint 64-row PE tiles; one exp covers
    both heads; out matmuls lag DEPTH steps and accumulate the softmax
    denominator via vh's ones column (host divides).
  - K-bias dropped (softmax-invariant), V-bias applied on host.
"""

import sys

sys.path.insert(0, "/opt/trn_rl_repo")

import os

import numpy as np

if os.environ.get("JAX_PLATFORMS") == "cpu":
    # the bass program must run on the neuron cores; the axon/neuron PJRT
    # platform registers only when JAX_PLATFORMS is unset/empty
    del os.environ["JAX_PLATFORMS"]

from collections import deque

import concourse.bass as bass  # noqa: F401
import concourse.mybir as mybir
from concourse import bacc
from concourse.tile import TileContext

B, S, E = 2, 2048, 1024
H, D = 16, 64
HPC = 4  # heads per core
COLS = HPC * D  # 256
P = 128
F32 = mybir.dt.float32
F16 = mybir.dt.float16
ET = E // P  # 8 e-tiles
JT = S // P  # 16 key tiles
NB = 512
NIQ = S // NB  # 4 query blocks
DEPTH1 = 6  # steady-state out-matmul lag (jt steps)
PE_BUFS = 30  # expT pool depth (peak backlog while iq0's outs wait on V)

_CACHED = {}


def build():
    nc = bacc.Bacc("TRN2", target_bir_lowering=False, debug=False)
    # host pre-arranges x as [p, et, 512] per 512-column chunk, one DRAM
    # tensor per chunk so every DMA is a fully contiguous sequential HBM
    # read (strided reads measured ~2x slower), and weights as
    # [p, et, 128] per chunk so ch0 can load first.
    CW = ET * NB  # x-chunk flat cols
    qTs = [nc.dram_tensor(f"qT{c}", [P, CW], F16, kind="ExternalInput")
           for c in range(NIQ)]
    kTs = [nc.dram_tensor(f"kT{c}", [P, CW], F16, kind="ExternalInput")
           for c in range(NIQ)]
    vTs = [nc.dram_tensor(f"vT{c}", [P, CW], F16, kind="ExternalInput")
           for c in range(NIQ)]
    wqs = [nc.dram_tensor(f"wq{c}", [P, ET * P], F16, kind="ExternalInput")
           for c in range(2)]
    wks = [nc.dram_tensor(f"wk{c}", [P, ET * P], F16, kind="ExternalInput")
           for c in range(2)]
    wv = nc.dram_tensor("wv", [P, ET * COLS], F16, kind="ExternalInput")
    bq = nc.dram_tensor("bq", [P, 2], F32, kind="ExternalInput")
    # out_raw[:, (h*NIQ+iq)*NB : ...]: rows 0-63 numerator (d), row 64 denom
    out_raw = nc.dram_tensor("out_raw", [65, HPC * S], F32,
                             kind="ExternalOutput")  # [65, 8192]

    with TileContext(nc) as tc:
        with (
            tc.tile_pool(name="wp", bufs=1) as wp,
            tc.tile_pool(name="xp", bufs=1) as xp,
            tc.tile_pool(name="hp", bufs=1) as hp,
            tc.tile_pool(name="pe", bufs=PE_BUFS) as pe,
            tc.tile_pool(name="ob", bufs=4) as ob,
            tc.tile_pool(name="psS", bufs=2, space="PSUM") as psS,
            tc.tile_pool(name="psO", bufs=2, space="PSUM") as psO,
        ):
            # --- weights wk/wq ch0 + bias first; rest lands just in time ---
            wq_b = wp.tile([P, 2, ET, P], F16)  # [p, ch, et, m]
            wk_b = wp.tile([P, 2, ET, P], F16)
            wv_b = wp.tile([P, ET, COLS], F16)
            bq_t = wp.tile([P, 2], F32)
            # x: [p, chunk(sb), et, 512]
            kxt = xp.tile([P, NIQ, ET, NB], F16)
            qxt = xp.tile([P, NIQ, ET, NB], F16)
            vxt = xp.tile([P, NIQ, ET, NB], F16)

            def x_chunk(dst, srcs, c, lo=0, hi=ET):
                nc.sync.dma_start(dst[:, c, lo:hi], srcs[c][:, lo * NB:hi * NB])

            nc.sync.dma_start(wk_b[:, 0], wks[0][:, :])
            nc.sync.dma_start(wq_b[:, 0], wqs[0][:, :])
            nc.sync.dma_start(bq_t, bq[:, :])
            x_chunk(kxt, kTs, 0, 0, 4)
            x_chunk(kxt, kTs, 0, 4, 8)
            x_chunk(qxt, qTs, 0, 0, 4)
            x_chunk(qxt, qTs, 0, 4, 8)
            x_chunk(kxt, kTs, 1)
            x_chunk(kxt, kTs, 2)
            x_chunk(kxt, kTs, 3)
            x_chunk(qxt, qTs, 1)
            nc.sync.dma_start(wv_b, wv[:, :])
            x_chunk(vxt, vTs, 0)
            x_chunk(vxt, vTs, 1)
            x_chunk(vxt, vTs, 2)
            x_chunk(vxt, vTs, 3)
            x_chunk(qxt, qTs, 2)
            x_chunk(qxt, qTs, 3)
            nc.sync.dma_start(wk_b[:, 1], wks[1][:, :])
            nc.sync.dma_start(wq_b[:, 1], wqs[1][:, :])

            # --- warm the exp table set early (hides ~2.7us ACT_TABLE_LOAD) ---
            warm = wp.tile([P, 2], F32)
            nc.scalar.activation(warm, bq_t,
                                 mybir.ActivationFunctionType.Exp, scale=0.0)

            # --- PE clock pre-warm: ~5us of dummy matmuls during the input
            # DMA wait lifts the HAM gate to 2.4 GHz before real work ---
            dum = wp.tile([P, NB], F16)
            nc.vector.memset(dum, 0.0)
            dps = psO.tile([P, NB], F32, tag="v", name="dps")
            for _ in range(12):
                nc.tensor.matmul(dps, dum[:, 0:P], dum, start=True, stop=True)

            # --- resident head tensors ---
            qhT = hp.tile([P, 2, S], F16)  # [2 heads x 64 d, chunk, s]
            khT = hp.tile([P, 2, S], F16)
            vh_aug = hp.tile([P, JT, HPC * 65], F16)
            vh_r = vh_aug.rearrange("p j (h x) -> p j h x", x=65)
            nc.vector.memset(vh_r[:, :, :, 64:65], 1.0)

            # --- projection emitters (generators yielding ~430ns chunks) ---
            def qk_proj_tile(tag, w_b, dst, bias, ch, sb):
                """One [128, 512] projection output tile: 8 matmuls + evac."""
                x = kxt if dst is khT else qxt
                ps = psO.tile([P, NB], F32, tag=tag, name=f"ps_{tag}")
                for et in range(ET):
                    nc.tensor.matmul(
                        ps,
                        w_b[:, ch, et, :],
                        x[:, sb, et, :],
                        start=(et == 0),
                        stop=(et == ET - 1),
                    )
                    if et % 2 == 1 and et < ET - 1:
                        yield
                dslc = dst[:, ch, sb * NB:(sb + 1) * NB]
                if bias is not None:
                    nc.vector.tensor_scalar_add(dslc, ps, bias[:, ch:ch + 1])
                else:
                    nc.vector.tensor_copy(dslc, ps)
                yield

            def v_proj_tile(scp):
                """V tiles for key tiles (2*scp, 2*scp+1): [128, 512] psum."""
                ps = psO.tile([P, NB], F32, tag="v", name="ps_v")
                for i in range(2):
                    sc = 2 * scp + i
                    for et in range(ET):
                        nc.tensor.matmul(
                            ps[:, i * COLS:(i + 1) * COLS],
                            vxt[:, sc // 4, et, (sc % 4) * P:(sc % 4 + 1) * P],
                            wv_b[:, et, :],
                            start=(et == 0),
                            stop=(et == ET - 1),
                        )
                        if et % 4 == 3 and (i, et) != (1, ET - 1):
                            yield
                for i in range(2):
                    sc = 2 * scp + i
                    nc.vector.tensor_copy(
                        vh_r[:, sc, :, :D],
                        ps[:, i * COLS:(i + 1) * COLS].rearrange(
                            "p (h d) -> p h d", d=D),
                    )
                yield

            # --- upfront: just K ch0 sb0 + Q ch0 sb0 (gates the first exp) ---
            for _ in qk_proj_tile("o", wk_b, khT, None, 0, 0):
                pass
            for _ in qk_proj_tile("o", wq_b, qhT, bq_t, 0, 0):
                pass

            # --- everything else is deferred, interleaved into attention as
            # small chunks (PE is in-order: a whole 1.7us tile between two
            # score pairs would stall the exp stream). slot = earliest step.
            def mk(kind, a, b):
                if kind == "v":
                    return v_proj_tile(a)
                if kind == "q":
                    return qk_proj_tile("v", wq_b, qhT, bq_t, a, b)
                return qk_proj_tile("v", wk_b, khT, None, a, b)

            deferred = deque(
                [(2, ("k", 0, 1)), (4, ("k", 0, 2)), (7, ("k", 0, 3)),
                 (9, ("q", 0, 1))]
                + [(15 + 3 * scp, ("v", scp, None)) for scp in range(8)]
                + [(40, ("q", 0, 2)), (43, ("q", 0, 3)),
                   (46, ("k", 1, 0)), (49, ("k", 1, 1)),
                   (52, ("k", 1, 2)), (55, ("k", 1, 3)),
                   (58, ("q", 1, 0)), (61, ("q", 1, 1)),
                   (64, ("q", 1, 2)), (67, ("q", 1, 3))]
            )
            live_gen = []

            def emit_deferred(it):
                # one ~430ns chunk per step keeps PE ahead of ScalarE
                if live_gen:
                    try:
                        next(live_gen[0])
                    except StopIteration:
                        live_gen.pop(0)
                    return
                if deferred and deferred[0][0] <= it:
                    _, (kind, a, b) = deferred.popleft()
                    live_gen.append(mk(kind, a, b))
                    emit_deferred(it)

            # --- attention ---
            pending = deque()  # (it, pr, iq, jt, expT)
            ops = {}

            def emit_out(pr, iq, jt, expT):
                op0, op1 = ops[(pr, iq)]
                for hh, op in ((0, op0), (1, op1)):
                    nc.tensor.matmul(
                        op[:65, :],
                        vh_r[:, jt, 2 * pr + hh, :],
                        expT[:, hh, :],
                        start=(jt == 0),
                        stop=(jt == JT - 1),
                    )
                if jt == JT - 1:  # evacuate + store this iq's outputs
                    for hh, op in ((0, op0), (1, op1)):
                        r = (2 * pr + hh) * NIQ + iq
                        osb = ob.tile([P, NB], F32, tag="ob", name="osb")
                        nc.vector.tensor_copy(osb[:65, :], op[:65, :])
                        nc.sync.dma_start(
                            out_raw[:, r * NB:(r + 1) * NB], osb[:65, :]
                        )
                    del ops[(pr, iq)]

            # out(pr0,iq0,jt) must wait for its V tile (DMA-gated); later
            # outs just lag DEPTH1 steps. Pops are capped at 2/step so a
            # backlog drains without swamping a single ACT period.
            def ready_step(it0, pr, iq, jt):
                if it0 < JT:
                    return 19 + 3 * (jt // 2)
                if it0 >= 112:
                    return it0 + 2
                return it0 + DEPTH1

            steps = [(pr, iq, jt) for pr in range(2) for iq in range(NIQ)
                     for jt in range(JT)]
            for it, (pr, iq, jt) in enumerate(steps):
                if jt == 0:
                    ops[(pr, iq)] = (
                        psO.tile([P, NB], F32, tag="o", name="op0"),
                        psO.tile([P, NB], F32, tag="o", name="op1"),
                    )
                sps = psS.tile([P, 2 * NB], F32, tag="s", name="sps")
                for hh in range(2):
                    r0 = hh * D
                    nc.tensor.matmul(
                        sps[:, hh * NB:(hh + 1) * NB],
                        khT[r0:r0 + D, pr, jt * P:(jt + 1) * P],
                        qhT[r0:r0 + D, pr, iq * NB:(iq + 1) * NB],
                        start=True,
                        stop=True,
                    )
                expT = pe.tile([P, 2, NB], F16, tag="e", name="expT")
                nc.scalar.activation(
                    expT, sps, mybir.ActivationFunctionType.Exp, scale=0.125
                )
                pending.append((it, pr, iq, jt, expT))
                max_pops = 1 if live_gen else (4 if it >= 120 else 2)
                for _ in range(max_pops):
                    if pending and ready_step(*pending[0][:3],
                                              pending[0][3]) <= it:
                        emit_out(*pending.popleft()[1:])
                    else:
                        break
                emit_deferred(it)
            while live_gen or deferred:
                emit_deferred(1 << 30)
            while pending:
                emit_out(*pending.popleft()[1:])
    nc.finalize()
    return nc


def _prep_in_maps(q, k, v, wq, bq, wk, bk, wv, bv):
    bf = np.float16
    q, k, v = (np.asarray(x, np.float32) for x in (q, k, v))
    wqb, wkb, wvb = (np.asarray(x, bf) for x in (wq, wk, wv))
    bq = np.asarray(bq, np.float32)

    def prep_x(x):
        # [S, E] -> per 512-col chunk: [p, et, nb] flattened to [P, ET*NB]
        a = x.astype(bf).reshape(NIQ, NB, ET, P).transpose(0, 3, 2, 1)
        return [np.ascontiguousarray(a[c].reshape(P, ET * NB))
                for c in range(NIQ)]

    def prep_wqk(w, cs):
        # [E, 256] -> per ch: [p, et, m] flattened
        a = w[:, cs].reshape(ET, P, 2, P).transpose(2, 1, 0, 3)
        return [np.ascontiguousarray(a[c].reshape(P, ET * P))
                for c in range(2)]

    def prep_wv(w, cs):
        a = w[:, cs].reshape(ET, P, COLS).transpose(1, 0, 2)
        return np.ascontiguousarray(a.reshape(P, ET * COLS))

    qT = [prep_x(q[b]) for b in range(B)]
    kT = [prep_x(k[b]) for b in range(B)]
    vT = [prep_x(v[b]) for b in range(B)]
    in_maps = []
    for c in range(8):
        b, g = divmod(c, 4)
        cs = slice(g * COLS, (g + 1) * COLS)
        m = {"bq": np.ascontiguousarray(bq[cs].reshape(2, P).T),
             "wv": prep_wv(wvb, cs)}
        for i in range(NIQ):
            m[f"qT{i}"] = qT[b][i]
            m[f"kT{i}"] = kT[b][i]
            m[f"vT{i}"] = vT[b][i]
        wql, wkl = prep_wqk(wqb, cs), prep_wqk(wkb, cs)
        for i in range(2):
            m[f"wq{i}"] = wql[i]
            m[f"wk{i}"] = wkl[i]
        in_maps.append(m)
    return in_maps


def _make_runner(nc, n_cores=8):
    """Persistent jitted shard_map runner over the prebuilt Bass module."""
    import jax
    from jax.experimental.shard_map import shard_map
    from jax.sharding import Mesh, NamedSharding, PartitionSpec
    from concourse import bass2jax

    bass2jax.install_neuronx_cc_hook()

    in_names, out_names, out_avals, zero_outs = [], [], [], []
    for alloc in nc.m.functions[0].allocations:
        if not isinstance(alloc, mybir.MemoryLocationSet):
            continue
        name = alloc.memorylocations[0].name
        if alloc.kind == "ExternalInput":
            in_names.append(name)
        elif alloc.kind == "ExternalOutput":
            shape = tuple(alloc.tensor_shape)
            dtype = mybir.dt.np(alloc.dtype)
            out_avals.append(jax.core.ShapedArray(shape, dtype))
            zero_outs.append(np.zeros((n_cores * shape[0], *shape[1:]), dtype))
            out_names.append(name)
    pid_name = nc.partition_id_tensor.name if nc.partition_id_tensor else None
    if pid_name is not None:
        in_names = [n for n in in_names if n != pid_name]
    n_params = len(in_names)
    all_names = in_names + out_names + ([pid_name] if pid_name else [])

    def _body(*args):
        operands = list(args)
        if pid_name is not None:
            operands.append(bass2jax.partition_id_tensor())
        outs = bass2jax._bass_exec_p.bind(
            *operands,
            out_avals=tuple(out_avals),
            in_names=tuple(all_names),
            out_names=tuple(out_names),
            lowering_input_output_aliases=(),
            sim_require_finite=True,
            sim_require_nnan=True,
            nc=nc,
        )
        return tuple(outs)

    devices = jax.devices()[:n_cores]
    mesh = Mesh(np.asarray(devices), ("core",))
    nio = n_params + len(out_names)
    sharded = jax.jit(
        shard_map(
            _body,
            mesh=mesh,
            in_specs=(PartitionSpec("core"),) * nio,
            out_specs=(PartitionSpec("core"),) * len(out_names),
            check_rep=False,
        ),
        keep_unused=True,
    )
    row_sharding = NamedSharding(mesh, PartitionSpec("core"))
    zeros_dev = [jax.device_put(z, row_sharding) for z in zero_outs]

    def run(in_maps):
        concat_in = [
            np.concatenate([np.asarray(m[name]) for m in in_maps], axis=0)
            for name in in_names
        ]
        out_arrs = sharded(*concat_in, *zeros_dev)
        return [
            {
                name: np.asarray(out_arrs[i]).reshape(n_cores, *out_avals[i].shape)[c]
                for i, name in enumerate(out_names)
            }
            for c in range(n_cores)
        ]

    run.sharded = sharded
    run.in_names = in_names
    run.zeros_dev = zeros_dev
    run.row_sharding = row_sharding
    return run


def get_runner():
    if "run" not in _CACHED:
        _CACHED["nc"] = build()
        _CACHED["run"] = _make_runner(_CACHED["nc"])
    return _CACHED["run"]


def kernel(q, k, v, wq, bq, wk, bk, wv, bv):
    run = get_runner()
    in_maps = _prep_in_maps(q, k, v, wq, bq, wk, bk, wv, bv)
    results = run(in_maps)

    bv = np.asarray(bv, np.float32)
    out = np.empty((B, S, E), np.float32)
    for c in range(8):
        b, g = divmod(c, 4)
        raw = results[c]["out_raw"]  # [65, 8192]
        num = raw[:64].reshape(64, HPC, S)  # [d, h, i] (NIQ*NB = S)
        den = raw[64].reshape(HPC, S)
        for h in range(HPC):
            col0 = g * COLS + h * D
            o = num[:, h, :] / den[h][None, :]
            out[b, :, col0:col0 + D] = o.T + bv[col0:col0 + D][None, :]
    return out
